# revision 16
# baseline (speedup 1.0000x reference)
"""Trainium2 Bass kernel for nn_CustomCLIP (moe_routing).

Reference computation (B=16384, C=512, H=128, D=3, n_text=1380):
    h_d  = relu(x @ W1[d])                  for d in 0..2      [D,B,H]
    a_d  = relu(h_d @ W2[d])                                   [D,B,C]
    ad   = a[label[b], b, :]                (per-sample routing)
    f    = 0.2*ad + 0.8*x ; f /= ||f||
    t    = txt / ||txt||  (rows)
    out  = exp(ls) * f @ t.T                                   [B, n_text]

Strategy: batch data-parallel over 8 cores (2048 rows each). Host passes
x pre-transposed and pre-scaled by 0.8 (W1 is divided by 0.8 to
compensate; relu commutes with positive scaling), W2 pre-scaled by 0.2,
and one-hot routing masks. The on-chip pipeline then runs entirely in
"feature dim on partitions" orientation so no on-chip transposes are
needed anywhere:

    hT_d   = relu(W1'[d].T @ xT') * bcast(mask_d)      [128h, b]
    aT     = relu(sum_d W2'[d].T @ hmT_d)              [512c, b]
    fT     = aT + xT'                                  (= feats.T)
    nsq_b  = (sq fT slice).T @ ones  -> [128b,1]  per-partition norms
    logits = fT_slice.T @ ttn  scaled by 1/||f|| on the PSUM->SBUF copy

exp(logit_scale) and 1/||txt_row|| are folded into the text features
once per kernel. All matmuls use float32r (TF32-style full-rate fp32
mode); fp32r operands are pre-rounded on the host (DMA inputs) or
rounded on write by the producing engine (declared fp32r out dtype).
"""

import numpy as np

import concourse.bass as bass
import concourse.bacc as bacc
import concourse.mybir as mybir
from concourse.tile import TileContext
from concourse.bass_utils import run_bass_kernel_spmd

F32 = mybir.dt.float32
F32R = mybir.dt.float32r
AF = mybir.ActivationFunctionType
ALU = mybir.AluOpType

B, C, H, D = 16384, 512, 128, 3
NT = (D + 1) * 345  # 1380
N_CORES = 8
BC = B // N_CORES   # 2048 rows per core
BF = 512            # batch free-dim chunk (one PSUM bank of fp32)
NCH = BC // BF      # 4 chunks per core
KC = C // 128       # 4 contraction chunks of 128
# text column ranges (<=512 per PSUM bank)
NRS = [(0, 512), (512, 1024), (1024, NT)]


def build_nc() -> bass.Bass:
    nc = bacc.Bacc(None, target_bir_lowering=False)

    xT = nc.dram_tensor("xt", [C, BC], F32R, kind="ExternalInput")
    mk = nc.dram_tensor("mk", [1, D, BC], F32R, kind="ExternalInput")
    w1 = nc.dram_tensor("w1", [D, C, H], F32R, kind="ExternalInput")
    w2 = nc.dram_tensor("w2", [D, H, C], F32R, kind="ExternalInput")
    tt = nc.dram_tensor("tt", [C, NT], F32R, kind="ExternalInput")
    ls = nc.dram_tensor("ls", [1, 1], F32, kind="ExternalInput")
    out = nc.dram_tensor("out", [BC, NT], F32, kind="ExternalOutput")

    xT_r = xT[:].rearrange("(kc p) b -> p kc b", p=128)
    w1_r = w1[:].rearrange("d (kc p) h -> p (d kc) h", p=128)
    w2_r = w2[:].rearrange("d p c -> p d c")
    tt_r = tt[:].rearrange("(kc p) n -> p kc n", p=128)

    with TileContext(nc) as tc:
        with (
            nc.allow_low_precision(reason="fp32r (tf32) matmul operands"),
            tc.tile_pool(name="cpool", bufs=1) as cpool,
            tc.tile_pool(name="xpool", bufs=2) as xpool,
            tc.tile_pool(name="mpool", bufs=2) as mpool,
            tc.tile_pool(name="hpool", bufs=6) as hpool,
            tc.tile_pool(name="fpool", bufs=2) as fpool,
            tc.tile_pool(name="sqpool", bufs=2) as sqpool,
            tc.tile_pool(name="rnpool", bufs=10) as rnpool,
            tc.tile_pool(name="opool", bufs=3) as opool,
            tc.tile_pool(name="ps", bufs=4, space="PSUM") as ps,
            tc.tile_pool(name="psl", bufs=4, space="PSUM") as psl,
        ):
            # ---- constants ----
            w1_sb = cpool.tile([128, D * KC, H], F32R)
            nc.sync.dma_start(out=w1_sb, in_=w1_r)
            w2_sb = cpool.tile([128, D, C], F32R)
            nc.sync.dma_start(out=w2_sb, in_=w2_r)
            tt_sb = cpool.tile([128, KC, NT], F32R)
            nc.sync.dma_start(out=tt_sb, in_=tt_r)
            ls_sb = cpool.tile([1, 1], F32)
            nc.sync.dma_start(out=ls_sb, in_=ls[:])
            # memset cannot encode fp32r values; stage in f32 and cast-copy
            ones_st = cpool.tile([128, 2], F32)
            nc.vector.memset(ones_st, 1.0)
            ones2 = cpool.tile([128, 2], F32R)
            nc.vector.tensor_copy(ones2, ones_st)
            ones_row_st = cpool.tile([1, 128], F32)
            nc.vector.memset(ones_row_st, 1.0)
            ones_row = cpool.tile([1, 128], F32R)
            nc.vector.tensor_copy(ones_row, ones_row_st)

            # ---- text prep: ttn = tt * (exp(ls) / ||tt_col||) in place ----
            s_sb = cpool.tile([1, 1], F32)
            nc.scalar.activation(out=s_sb, in_=ls_sb, func=AF.Exp)
            rnt = cpool.tile([1, NT], F32R)
            nsq_t = []
            for n0, n1 in NRS:
                t_ps = ps.tile([1, 512], F32, tag="ps", name=f"tps_{n0}")
                nsq_t.append(t_ps)
            for kc in range(KC):
                tq = sqpool.tile([128, NT], F32R, tag="tq")
                nc.scalar.activation(out=tq, in_=tt_sb[:, kc, :], func=AF.Square)
                for i, (n0, n1) in enumerate(NRS):
                    nc.tensor.matmul(
                        nsq_t[i][:, : n1 - n0],
                        ones2[:, 0:1],
                        tq[:, n0:n1],
                        start=(kc == 0),
                        stop=(kc == KC - 1),
                    )
            for i, (n0, n1) in enumerate(NRS):
                nc.scalar.activation(
                    out=rnt[:, n0:n1], in_=nsq_t[i][:, : n1 - n0], func=AF.Sqrt
                )
            nc.vector.reciprocal(out=rnt, in_=rnt)
            nc.vector.tensor_scalar_mul(rnt, rnt, s_sb[0:1, 0:1])
            for i, (n0, n1) in enumerate(NRS):
                rbt = psl.tile([128, 512], F32, tag="pl", name=f"rbt_{n0}")
                nc.tensor.matmul(
                    rbt[:, : n1 - n0],
                    ones_row,
                    rnt[:, n0:n1],
                    start=True,
                    stop=True,
                )
                for kc in range(KC):
                    nc.vector.tensor_mul(
                        tt_sb[:, kc, n0:n1], tt_sb[:, kc, n0:n1], rbt[:, : n1 - n0]
                    )

            # ---- main loop over batch chunks of BF ----
            for ch in range(NCH):
                b0 = ch * BF
                xt = xpool.tile([128, KC, BF], F32R)
                nc.sync.dma_start(out=xt, in_=xT_r[:, :, b0 : b0 + BF])
                mrow = mpool.tile([1, D, BF], F32R)
                nc.sync.dma_start(out=mrow, in_=mk[:, :, b0 : b0 + BF])

                # stage 1: hm_d = relu(W1'[d].T @ xT) * mask_d
                hms = []
                for d in range(D):
                    mb = ps.tile([128, BF], F32, tag="ps", name=f"mb_{ch}_{d}")
                    nc.tensor.matmul(
                        mb, ones_row, mrow[:, d, :], start=True, stop=True
                    )
                    h = ps.tile([128, BF], F32, tag="ps", name=f"h_{ch}_{d}")
                    for kc in range(KC):
                        nc.tensor.matmul(
                            h,
                            w1_sb[:, d * KC + kc, :],
                            xt[:, kc, :],
                            start=(kc == 0),
                            stop=(kc == KC - 1),
                        )
                    rh = hpool.tile([128, BF], F32, tag="rh", name=f"rh_{ch}_{d}")
                    nc.scalar.activation(out=rh, in_=h, func=AF.Relu)
                    hm = hpool.tile([128, BF], F32R, tag="hm", name=f"hm_{ch}_{d}")
                    nc.vector.tensor_mul(hm, rh, mb)
                    hms.append(hm)

                # stage 2: fT_c = relu(sum_d W2'[d].T @ hm_d) + xT_c
                f = fpool.tile([128, KC, BF], F32R)
                for cc in range(KC):
                    a = ps.tile([128, BF], F32, tag="ps", name=f"a_{ch}_{cc}")
                    for d in range(D):
                        nc.tensor.matmul(
                            a,
                            w2_sb[:, d, cc * 128 : (cc + 1) * 128],
                            hms[d],
                            start=(d == 0),
                            stop=(d == D - 1),
                        )
                    ra = hpool.tile([128, BF], F32, tag="ra", name=f"ra_{ch}_{cc}")
                    nc.scalar.activation(out=ra, in_=a, func=AF.Relu)
                    nc.vector.tensor_add(f[:, cc, :], ra, xt[:, cc, :])

                # norms: rn_bs = 1/||f_row|| as [128,1] per 128-row subchunk
                sq = sqpool.tile([128, KC, BF], F32R, tag="sq")
                for cc in range(KC):
                    nc.scalar.activation(
                        out=sq[:, cc, :], in_=f[:, cc, :], func=AF.Square
                    )
                rns = []
                for bs in range(4):
                    # N=2 (duplicated column): fp32r matmuls reject N=1
                    nsq = ps.tile([128, 2], F32, tag="ps", name=f"nsq_{ch}_{bs}")
                    for cc in range(KC):
                        nc.tensor.matmul(
                            nsq,
                            sq[:, cc, bs * 128 : (bs + 1) * 128],
                            ones2,
                            start=(cc == 0),
                            stop=(cc == KC - 1),
                        )
                    sn = rnpool.tile([128, 1], F32, tag="sn", name=f"sn_{ch}_{bs}")
                    nc.scalar.activation(out=sn, in_=nsq[:, 0:1], func=AF.Sqrt)
                    rn = rnpool.tile([128, 1], F32, tag="rn", name=f"rn_{ch}_{bs}")
                    nc.vector.reciprocal(out=rn, in_=sn)
                    rns.append(rn)

                # logits: out_rows = (fT_slice.T @ ttn) * rn_bs
                for bs in range(4):
                    lps = []
                    for i, (n0, n1) in enumerate(NRS):
                        lp = psl.tile([128, 512], F32, tag="pl", name=f"lp_{ch}_{bs}_{i}")
                        lps.append(lp)
                    for kc in range(KC):
                        for i, (n0, n1) in enumerate(NRS):
                            nc.tensor.matmul(
                                lps[i][:, : n1 - n0],
                                f[:, kc, bs * 128 : (bs + 1) * 128],
                                tt_sb[:, kc, n0:n1],
                                start=(kc == 0),
                                stop=(kc == KC - 1),
                            )
                    ob = opool.tile([128, NT], F32)
                    for i, (n0, n1) in enumerate(NRS):
                        nc.any.tensor_scalar_mul(
                            ob[:, n0:n1], lps[i][:, : n1 - n0], rns[bs]
                        )
                    r0 = b0 + bs * 128
                    nc.sync.dma_start(out=out[r0 : r0 + 128, :], in_=ob)

    nc.compile()
    return nc


_NC_CACHE: list = []


def _get_nc() -> bass.Bass:
    if not _NC_CACHE:
        _NC_CACHE.append(build_nc())
    return _NC_CACHE[0]


def _tf32_round(a: np.ndarray) -> np.ndarray:
    """Round fp32 to the fp32r/tf32 grid (10-bit mantissa, RNE)."""
    u = np.ascontiguousarray(a, dtype=np.float32).view(np.uint32)
    lsb = (u >> 13) & 1
    rounded = (u + 0x0FFF + lsb) & np.uint32(0xFFFFE000)
    return rounded.view(np.float32)


def make_in_maps(
    image_features: np.ndarray,
    domain_label: np.ndarray,
    W1: np.ndarray,
    W2: np.ndarray,
    text_features: np.ndarray,
    logit_scale: np.ndarray,
) -> list[dict[str, np.ndarray]]:
    x = np.asarray(image_features, dtype=np.float32)
    lab = np.asarray(domain_label).astype(np.int64)
    w1 = np.asarray(W1, dtype=np.float32)
    w2 = np.asarray(W2, dtype=np.float32)
    txt = np.asarray(text_features, dtype=np.float32)
    lsv = np.asarray(logit_scale, dtype=np.float32).reshape(1, 1)

    xT = _tf32_round((x * np.float32(0.8)).T)                   # [C, B]
    mk = (lab[None, :] == np.arange(D)[:, None]).astype(np.float32)  # [D, B]
    w1s = _tf32_round(w1 / np.float32(0.8))
    w2s = _tf32_round(w2 * np.float32(0.2))
    ttT = _tf32_round(txt.T)                                    # [C, NT]

    in_maps = []
    for c in range(N_CORES):
        sl = slice(c * BC, (c + 1) * BC)
        in_maps.append(
            {
                "xt": np.ascontiguousarray(xT[:, sl]),
                "mk": np.ascontiguousarray(mk[:, sl])[None],
                "w1": w1s,
                "w2": w2s,
                "tt": ttT,
                "ls": lsv,
            }
        )
    return in_maps


def kernel(
    image_features: np.ndarray,
    domain_label: np.ndarray,
    W1: np.ndarray,
    W2: np.ndarray,
    text_features: np.ndarray,
    logit_scale: np.ndarray,
) -> np.ndarray:
    nc = _get_nc()
    in_maps = make_in_maps(
        image_features, domain_label, W1, W2, text_features, logit_scale
    )
    res = run_bass_kernel_spmd(nc, in_maps, list(range(N_CORES)))
    return np.concatenate([r["out"] for r in res.results], axis=0)
